# revision 21
# baseline (speedup 1.0000x reference)
"""Trainium2 Bass kernel for nn_AttentionFusion (cross-attention, B=4, LQ=1024,
LKV=4096, D=512, H=4 heads of 128).

Sharding: 8 cores = (batch b in 0..3) x (head-pair hp in 0..1). Core c = 2*b+hp
computes attention for heads {2hp, 2hp+1} of batch b plus its partial
out-projection (tensor-parallel split of Wo). Host sums the two partials per
batch (the TP un-shard).

x and enc are transposed + cast to bf16 on the HOST, so the device loads
xT/eT directly (contiguous DMA) and spends zero PE time on input transposes.
(fp8 projections were tried and rejected: independent per-kv fp8 noise on
scores does not average out relative to ctx's own random-sum magnitude, so
final error tracks the ~7% score noise — over the 2e-2 budget.) bk is dropped
(softmax is invariant to a per-query constant), bv is folded into cvec.

v4 structure: heads run SEQUENTIALLY (h0 then h1): PSUM = 3-deep scores
rotation (6 banks) + 1 ctx accumulator (2 banks). All projections are
injected between h0's attention steps; 8 of h1's score/exp tiles are
precomputed ("prefetched") late in the h0 phase into a stash pool so the
ACT-bound h1 phase shortens; h0's finish + out-projection inject into h1's
early steps. ctx matmuls trail their exp by 2 steps. GpSimd stays off bulk
work (SBUF port contention halves DVE throughput).
"""

import numpy as np

B, LQ, LKV, D, H, HD = 4, 1024, 4096, 512, 4, 128
NCORES = 8
SCALE = 1.0 / float(np.sqrt(HD))

_compiled = {}


def _build():
    import concourse.bacc as bacc
    import concourse.mybir as mybir
    from concourse import tile
    from concourse.masks import make_identity

    bf16, f32 = mybir.dt.bfloat16, mybir.dt.float32
    EXP = mybir.ActivationFunctionType.Exp
    IDN = mybir.ActivationFunctionType.Identity

    nc = bacc.Bacc(
        "TRN2",
        target_bir_lowering=False,
        debug=False,
        enable_asserts=True,
        num_devices=NCORES,
    )

    et = nc.dram_tensor("et", [512, LKV], bf16, kind="ExternalInput")
    xt = nc.dram_tensor("xt", [512, LQ], bf16, kind="ExternalInput")
    wqt = nc.dram_tensor("wqt", [128, 1024], bf16, kind="ExternalInput")
    wkt = nc.dram_tensor("wkt", [128, 1024], bf16, kind="ExternalInput")
    wvt = nc.dram_tensor("wvt", [128, 1024], bf16, kind="ExternalInput")
    wot = nc.dram_tensor("wot", [128, 1024], bf16, kind="ExternalInput")
    bq2 = nc.dram_tensor("bq2", [128, 2], f32, kind="ExternalInput")
    cvec = nc.dram_tensor("cvec", [D], f32, kind="ExternalInput")
    outp = nc.dram_tensor("outp", [LQ, D], f32, kind="ExternalOutput")

    with tile.TileContext(nc) as tc:
        with (
            tc.tile_pool(name="const", bufs=1) as const,
            tc.tile_pool(name="big", bufs=1) as big,
            tc.tile_pool(name="expp", bufs=6) as expp,
            tc.tile_pool(name="stash", bufs=8) as stashp,
            tc.tile_pool(name="tree", bufs=9) as treep,
            tc.tile_pool(name="smal", bufs=4) as smal,
            tc.tile_pool(name="nrm0p", bufs=8) as nrm0p,
            tc.tile_pool(name="osb", bufs=4) as osb,
            tc.tile_pool(name="ps", bufs=3, space="PSUM") as psp,
            tc.tile_pool(name="ps_c", bufs=1, space="PSUM") as ps_c,
        ):
            # --- DMAs, single sync ring, in consumption order ---
            bqsb = const.tile([128, 2], f32)
            nc.sync.dma_start(bqsb[:], bq2[:])
            wq_sb = const.tile([128, 4, 256], bf16)
            nc.sync.dma_start(wq_sb[:], wqt.ap().rearrange("p (k d) -> p k d", k=4))
            xT = big.tile([128, 4, LQ], bf16)
            nc.sync.dma_start(xT[:], xt.ap().rearrange("(k p) q -> p k q", p=128))
            wk_sb = const.tile([128, 4, 256], bf16)
            nc.sync.dma_start(wk_sb[:], wkt.ap().rearrange("p (k d) -> p k d", k=4))
            wv_sb = const.tile([128, 4, 256], bf16)
            nc.sync.dma_start(wv_sb[:], wvt.ap().rearrange("p (k d) -> p k d", k=4))
            eT = [big.tile([128, 4, 1024], bf16, name=f"eT{g}") for g in range(4)]
            for g in range(4):
                nc.sync.dma_start(
                    eT[g][:],
                    et.ap()[:, 1024 * g : 1024 * (g + 1)].rearrange(
                        "(k p) q -> p k q", p=128
                    ),
                )
            wo_sb = const.tile([128, 2, D], bf16)
            nc.sync.dma_start(wo_sb[:], wot.ap().rearrange("p (k d) -> p k d", k=2))

            # --- constants ---
            ones = const.tile([128, 1], f32)
            nc.vector.memset(ones[:], 1.0)
            identb = const.tile([128, 128], bf16)
            make_identity(nc, identb[:])
            # warm the ACT exp table set early (~2.7us table load)
            warm = const.tile([128, 1], f32)
            nc.scalar.activation(warm[:], ones[:], EXP)
            # cvec broadcast (needed mid-stream for the nrm0 adds)
            cvst = const.tile([128, D], f32)
            nc.sync.dma_start(cvst[0:1, :], cvec.ap().unsqueeze(0))
            cvsb = const.tile([128, D], f32)
            nc.gpsimd.partition_broadcast(cvsb[:], cvst[0:1, :])

            qT = [big.tile([128, LQ], bf16, name=f"qT{h}") for h in range(2)]
            kT = [
                [big.tile([128, 1024], bf16, name=f"kT{h}_{g}") for g in range(4)]
                for h in range(2)
            ]
            v_g = [big.tile([128, 8, 256], bf16, name=f"v{g}") for g in range(4)]

            # --- projection units ---
            def unit_q(t):
                ps = psp.tile([128, 1024], f32, name=f"q_ps{t}", tag="sc")
                for c in range(2):
                    for k in range(4):
                        nc.tensor.matmul(
                            ps[:, 512 * c : 512 * c + 512],
                            wq_sb[:, k, 128 * t : 128 * t + 128],
                            xT[:, k, 512 * c : 512 * c + 512],
                            start=(k == 0),
                            stop=(k == 3),
                        )
                nc.scalar.activation(qT[t][:], ps[:], IDN, bias=bqsb[:, t : t + 1])

            def unit_k(h, g):
                ps = psp.tile([128, 1024], f32, name=f"k_ps{h}{g}", tag="sc")
                for c in range(2):
                    for k in range(4):
                        nc.tensor.matmul(
                            ps[:, 512 * c : 512 * c + 512],
                            wk_sb[:, k, 128 * h : 128 * h + 128],
                            eT[g][:, k, 512 * c : 512 * c + 512],
                            start=(k == 0),
                            stop=(k == 3),
                        )
                # h1's kT copies land in the ACT-slack h0 phase
                if h == 0:
                    nc.vector.tensor_copy(kT[h][g][:], ps[:])
                else:
                    nc.scalar.activation(kT[h][g][:], ps[:], IDN)

            def unit_v(g, pair):
                ps = psp.tile([128, 1024], f32, name=f"v_ps{g}{pair}", tag="sc")
                for w in range(2):
                    i = 2 * pair + w
                    for k in range(4):
                        nc.tensor.matmul(
                            ps[:, 256 * w : 256 * w + 256],
                            eT[g][:, k, 128 * i : 128 * i + 128],
                            wv_sb[:, k, :],
                            start=(k == 0),
                            stop=(k == 3),
                        )
                nc.vector.tensor_copy(
                    v_g[g][:, 2 * pair : 2 * pair + 2, :],
                    ps[:, 0:512].rearrange("p (w d) -> p w d", w=2),
                )

            # --- attention ---
            ctxT = big.tile([128, 2, LQ], bf16)
            att = {}
            recips = {}
            nrm0 = []
            out_ap = outp.ap().rearrange("(j p) e -> p j e", p=128)
            uid = [0]
            ESC = SCALE

            def _tr():
                uid[0] += 1
                return treep.tile([128, LQ], bf16, name=f"tr{uid[0]}", tag="tr")

            def _st(h):
                if h not in att:
                    att[h] = {
                        "ps_ctx": None,
                        "levels": [None] * 6,
                        "pend": [],
                        "run": None,
                        "npush": 0,
                    }
                return att[h]

            def tree_push(h, et_t):
                st = att[h]
                st["npush"] += 1
                if st["run"] is not None:
                    nxt = _tr()
                    nc.vector.tensor_add(nxt[:], st["run"][:], et_t[:])
                    st["run"] = nxt
                    return
                levels = st["levels"]
                cur, lvl = et_t, 0
                while levels[lvl] is not None:
                    nxt = _tr()
                    nc.vector.tensor_add(nxt[:], levels[lvl][:], cur[:])
                    levels[lvl] = None
                    cur, lvl = nxt, lvl + 1
                levels[lvl] = cur
                if st["npush"] == 25:
                    # collapse the tree into a running sum for a short tail
                    run = None
                    for l in range(6):
                        if levels[l] is None:
                            continue
                        if run is None:
                            run = levels[l]
                        else:
                            nxt = _tr()
                            nc.vector.tensor_add(nxt[:], run[:], levels[l][:])
                            run = nxt
                        levels[l] = None
                    st["run"] = run

            def emit_ctx_oldest(h, flush=False, maxpop=2, mindepth=2):
                st = _st(h)
                npop = 0
                while len(st["pend"]) > (0 if flush else mindepth) and (
                    flush or npop < maxpop
                ):
                    kt, et_t, g, i = st["pend"].pop(0)
                    npop += 1
                    if st["ps_ctx"] is None:
                        st["ps_ctx"] = ps_c.tile(
                            [128, LQ], f32, name=f"ctx{h}", tag="ctx"
                        )
                    for c in range(2):
                        nc.tensor.matmul(
                            st["ps_ctx"][:, 512 * c : 512 * c + 512],
                            v_g[g][:, i, 128 * h : 128 * h + 128],
                            et_t[:, 512 * c : 512 * c + 512],
                            start=(kt == 0),
                            stop=(kt == 31),
                        )
                    if kt != 31:
                        tree_push(h, et_t)
                    else:
                        st["last_et"] = et_t

            def score_exp(h, kt, pool):
                st = _st(h)
                g, i = kt // 8, kt % 8
                ps_sc = psp.tile([128, LQ], f32, name=f"sc{h}_{kt}", tag="sc")
                for c in range(2):
                    nc.tensor.matmul(
                        ps_sc[:, 512 * c : 512 * c + 512],
                        kT[h][g][:, 128 * i : 128 * i + 128],
                        qT[h][:, 512 * c : 512 * c + 512],
                        start=True,
                        stop=True,
                    )
                et_t = pool.tile([128, LQ], bf16, name=f"et{h}_{kt}", tag="et")
                nc.scalar.activation(et_t[:], ps_sc[:], EXP, scale=ESC)
                st["pend"].append((kt, et_t, g, i))

            def attn_step(h, kt):
                score_exp(h, kt, expp)
                # drain the backlog to depth 1 over the last steps so the
                # finish flush (and the tail's serial DVE chain) stays short —
                # a >3.4us PE idle there also re-throttles HAM for the tail MMs
                if h == 1 and kt >= 29:
                    emit_ctx_oldest(h, maxpop=3, mindepth=1)
                else:
                    emit_ctx_oldest(h)

            def finish_a(h):
                st = att[h]
                emit_ctx_oldest(h, flush=True)
                # ctxT halves first: they gate the tail out-projection MMs
                for c in range(2):
                    nc.vector.tensor_copy(
                        ctxT[:, h, 512 * c : 512 * c + 512],
                        st["ps_ctx"][:, 512 * c : 512 * c + 512],
                    )
                fin = _tr()
                for c in range(2):
                    nc.vector.tensor_add(
                        fin[:, 512 * c : 512 * c + 512],
                        st["run"][:, 512 * c : 512 * c + 512],
                        st["last_et"][:, 512 * c : 512 * c + 512],
                    )
                st["fin"] = fin

            def finish_b(h):
                st = att[h]
                fin = st["fin"]
                den = smal.tile([128, 8], f32, name=f"den{h}", tag="den")
                pt = psp.tile([128, LQ], bf16, name=f"dt{h}", tag="sc")
                for half in range(2):
                    for j in range(4):
                        jj = 4 * half + j
                        nc.tensor.transpose(
                            pt[:, 128 * jj : 128 * jj + 128],
                            fin[:, 128 * jj : 128 * jj + 128],
                            identb[:],
                        )
                    nc.vector.tensor_reduce(
                        den[:, 4 * half : 4 * half + 4],
                        pt[:, 512 * half : 512 * half + 512].rearrange(
                            "p (j q) -> p j q", j=4
                        ),
                        axis=mybir.AxisListType.X,
                        op=mybir.AluOpType.add,
                    )
                rc = smal.tile([128, 8], f32, name=f"rc{h}", tag="rc")
                nc.vector.reciprocal(rc[:], den[:])
                recips[h] = rc

            def outproj0(js):
                for j in js:
                    p = psp.tile([128, LQ], f32, name=f"o_ps0_{j}", tag="sc")
                    nc.tensor.matmul(
                        p[:, 0:512],
                        ctxT[:, 0, 128 * j : 128 * j + 128],
                        wo_sb[:, 0, :],
                        start=True,
                        stop=True,
                    )
                    n = nrm0p.tile([128, 512], f32, name=f"nrm0_{j}", tag="nrm0")
                    nc.vector.tensor_scalar_mul(
                        n[:], p[:, 0:512], recips[0][:, j : j + 1]
                    )
                    nc.vector.tensor_add(n[:], n[:], cvsb[:])
                    nrm0.append(n)

            def outproj1(js):
                for j in js:
                    p = psp.tile([128, LQ], f32, name=f"o_ps1_{j}", tag="sc")
                    nc.tensor.matmul(
                        p[:, 0:512],
                        ctxT[:, 1, 128 * j : 128 * j + 128],
                        wo_sb[:, 1, :],
                        start=True,
                        stop=True,
                    )
                    n1 = osb.tile([128, 512], f32, name=f"nrm1_{j}", tag="nrm1")
                    if j % 2 == 0:
                        nc.scalar.activation(
                            n1[:], p[:, 0:512], IDN, scale=recips[1][:, j : j + 1]
                        )
                    else:
                        nc.vector.tensor_scalar_mul(
                            n1[:], p[:, 0:512], recips[1][:, j : j + 1]
                        )
                    ob = osb.tile([128, 512], f32, name=f"ob{j}", tag="ob")
                    nc.vector.tensor_add(ob[:], nrm0[j][:], n1[:])
                    nc.sync.dma_start(out_ap[:, j, :], ob[:])

            # --- schedule ---
            inj = {}

            def add_inj(s, fn):
                inj.setdefault(s, []).append(fn)

            for gi, gn in enumerate((1, 2, 3)):
                base = 8 * gi
                add_inj(base + 0, lambda gn=gn: unit_k(0, gn))
                for pr in range(4):
                    add_inj(base + 1 + pr, lambda gn=gn, pr=pr: unit_v(gn, pr))
            add_inj(5, lambda: unit_k(1, 0))
            add_inj(13, lambda: unit_k(1, 1))
            add_inj(21, lambda: unit_k(1, 2))
            add_inj(26, lambda: unit_k(1, 3))
            # prefetch h1 kt0..7 score/exp into the late h0 phase
            for p in range(8):
                add_inj(24 + p, lambda p=p: score_exp(1, p, stashp))
            # finish_a(0) must be emitted BEFORE h1's first ctx matmul: ctx1's
            # PSUM buffer WAR-depends on ctx0's readers (the ctxT copies), and
            # the PE queue is strict FIFO.
            preinj = {32: [lambda: finish_a(0)]}
            add_inj(34, lambda: finish_b(0))
            # spread h0's out-projection: its 4 DVE ops per pair otherwise
            # congest the vector queue, delaying h1's tree pushes (et-pool
            # WAR -> exp stall -> PE gap)
            add_inj(36, lambda: outproj0([0, 1]))
            add_inj(40, lambda: outproj0([2, 3]))
            add_inj(44, lambda: outproj0([4, 5]))
            add_inj(48, lambda: outproj0([6, 7]))

            # pre-units: q projections + group-0 k/v
            unit_q(0)
            unit_q(1)
            unit_k(0, 0)
            unit_v(0, 0)
            unit_v(0, 1)
            unit_v(0, 2)
            unit_v(0, 3)

            for s in range(56):
                for fn in preinj.get(s, []):
                    fn()
                if s < 32:
                    attn_step(0, s)
                else:
                    attn_step(1, (s - 32) + 8)
                for fn in inj.get(s, []):
                    fn()

            finish_a(1)
            finish_b(1)
            outproj1(list(range(8)))

    nc.compile()
    return nc


def _get_nc():
    if "nc" not in _compiled:
        _compiled["nc"] = _build()
    return _compiled["nc"]


def _warr(wt, k, dtype_name="bfloat16", scale=1.0):
    """[k*128, n] -> [128, k*n] so partition p reads one contiguous block."""
    import ml_dtypes

    dt = getattr(ml_dtypes, dtype_name)
    n = wt.shape[1]
    return np.ascontiguousarray(
        (wt * scale).reshape(k, 128, n).transpose(1, 0, 2).reshape(128, k * n)
    ).astype(dt)


def _make_in_maps(x, encoder_feats, Wq, Wk, Wv, bq, bk, bv, Wo, bo):
    import ml_dtypes

    f = np.float32
    bf = ml_dtypes.bfloat16
    x = np.asarray(x, f)
    encoder_feats = np.asarray(encoder_feats, f)
    Wq, Wk, Wv, Wo = (np.asarray(a, f) for a in (Wq, Wk, Wv, Wo))
    bq, bk, bv, bo = (np.asarray(a, f) for a in (bq, bk, bv, bo))

    # host-side transpose + bf16 cast (one copy per batch)
    eT_b = [encoder_feats[b].T.astype(bf) for b in range(B)]  # [512, 4096] bf16
    xT_b = [x[b].T.astype(bf) for b in range(B)]  # [512, 1024] bf16

    # bk is dropped: adding bk to k shifts every score for a given query by the
    # same constant (q . bk), and softmax is invariant to that shift.
    per_hp = []
    for hp in range(2):
        sl = slice(256 * hp, 256 * hp + 256)
        cv = Wo[:, sl] @ bv[sl]
        if hp == 0:
            cv = cv + bo
        per_hp.append(
            {
                "wqt": _warr(Wq[sl, :].T, 4),
                "wkt": _warr(Wk[sl, :].T, 4),
                "wvt": _warr(Wv[sl, :].T, 4),
                "wot": _warr(Wo[:, sl].T, 2),
                "bq2": np.ascontiguousarray(bq[sl].reshape(2, 128).T, dtype=f),
                "cvec": np.ascontiguousarray(cv, dtype=f),
            }
        )

    in_maps = []
    for c in range(NCORES):
        b, hp = c // 2, c % 2
        m = {"et": eT_b[b], "xt": xT_b[b]}
        m.update(per_hp[hp])
        in_maps.append(m)
    return in_maps


def kernel(x, encoder_feats, Wq, Wk, Wv, bq, bk, bv, Wo, bo, _trace=False):
    from concourse.bass_utils import run_bass_kernel_spmd

    nc = _get_nc()
    in_maps = _make_in_maps(x, encoder_feats, Wq, Wk, Wv, bq, bk, bv, Wo, bo)
    kw = {}
    if _trace:
        kw = dict(trace=True, trace_cores=[0])
    res = run_bass_kernel_spmd(nc, in_maps, core_ids=list(range(NCORES)), **kw)
    _compiled["last_res"] = res
    out = np.empty((B, LQ, D), np.float32)
    for b in range(B):
        out[b] = res.results[2 * b]["outp"] + res.results[2 * b + 1]["outp"]
    return out


# revision 22
# speedup vs baseline: 1.0135x; 1.0135x over previous
"""Trainium2 Bass kernel for nn_AttentionFusion (cross-attention, B=4, LQ=1024,
LKV=4096, D=512, H=4 heads of 128).

Sharding: 8 cores = (batch b in 0..3) x (head-pair hp in 0..1). Core c = 2*b+hp
computes attention for heads {2hp, 2hp+1} of batch b plus its partial
out-projection (tensor-parallel split of Wo). Host sums the two partials per
batch (the TP un-shard).

x and enc are transposed + cast to bf16 on the HOST, so the device loads
xT/eT directly (contiguous DMA) and spends zero PE time on input transposes.
(fp8 projections were tried and rejected: independent per-kv fp8 noise on
scores does not average out relative to ctx's own random-sum magnitude, so
final error tracks the ~7% score noise — over the 2e-2 budget.) bk is dropped
(softmax is invariant to a per-query constant), bv is folded into cvec.

v4 structure: heads run SEQUENTIALLY (h0 then h1): PSUM = 3-deep scores
rotation (6 banks) + 1 ctx accumulator (2 banks). All projections are
injected between h0's attention steps; 8 of h1's score/exp tiles are
precomputed ("prefetched") late in the h0 phase into a stash pool so the
ACT-bound h1 phase shortens; h0's finish + out-projection inject into h1's
early steps. ctx matmuls trail their exp by 2 steps. GpSimd stays off bulk
work (SBUF port contention halves DVE throughput).
"""

import numpy as np

B, LQ, LKV, D, H, HD = 4, 1024, 4096, 512, 4, 128
NCORES = 8
SCALE = 1.0 / float(np.sqrt(HD))

_compiled = {}


def _build():
    import concourse.bacc as bacc
    import concourse.mybir as mybir
    from concourse import tile
    from concourse.masks import make_identity

    bf16, f32 = mybir.dt.bfloat16, mybir.dt.float32
    EXP = mybir.ActivationFunctionType.Exp
    IDN = mybir.ActivationFunctionType.Identity

    nc = bacc.Bacc(
        "TRN2",
        target_bir_lowering=False,
        debug=False,
        enable_asserts=True,
        num_devices=NCORES,
    )

    et = nc.dram_tensor("et", [512, LKV], bf16, kind="ExternalInput")
    xt = nc.dram_tensor("xt", [512, LQ], bf16, kind="ExternalInput")
    wqt = nc.dram_tensor("wqt", [128, 1024], bf16, kind="ExternalInput")
    wkt = nc.dram_tensor("wkt", [128, 1024], bf16, kind="ExternalInput")
    wvt = nc.dram_tensor("wvt", [128, 1024], bf16, kind="ExternalInput")
    wot = nc.dram_tensor("wot", [128, 1024], bf16, kind="ExternalInput")
    bq2 = nc.dram_tensor("bq2", [128, 2], f32, kind="ExternalInput")
    cvec = nc.dram_tensor("cvec", [D], f32, kind="ExternalInput")
    outp = nc.dram_tensor("outp", [LQ, D], f32, kind="ExternalOutput")

    with tile.TileContext(nc) as tc:
        with (
            tc.tile_pool(name="const", bufs=1) as const,
            tc.tile_pool(name="big", bufs=1) as big,
            tc.tile_pool(name="expp", bufs=6) as expp,
            tc.tile_pool(name="stash", bufs=8) as stashp,
            tc.tile_pool(name="tree", bufs=9) as treep,
            tc.tile_pool(name="smal", bufs=4) as smal,
            tc.tile_pool(name="nrm0p", bufs=8) as nrm0p,
            tc.tile_pool(name="osb", bufs=4) as osb,
            tc.tile_pool(name="ps", bufs=3, space="PSUM") as psp,
            tc.tile_pool(name="ps_c", bufs=1, space="PSUM") as ps_c,
        ):
            # --- DMAs, single sync ring, in consumption order ---
            bqsb = const.tile([128, 2], f32)
            nc.sync.dma_start(bqsb[:], bq2[:])
            wq_sb = const.tile([128, 4, 256], bf16)
            nc.sync.dma_start(wq_sb[:], wqt.ap().rearrange("p (k d) -> p k d", k=4))
            xT = big.tile([128, 4, LQ], bf16)
            nc.sync.dma_start(xT[:], xt.ap().rearrange("(k p) q -> p k q", p=128))
            wk_sb = const.tile([128, 4, 256], bf16)
            nc.sync.dma_start(wk_sb[:], wkt.ap().rearrange("p (k d) -> p k d", k=4))
            wv_sb = const.tile([128, 4, 256], bf16)
            nc.sync.dma_start(wv_sb[:], wvt.ap().rearrange("p (k d) -> p k d", k=4))
            eT = [big.tile([128, 4, 1024], bf16, name=f"eT{g}") for g in range(4)]
            for g in range(4):
                nc.sync.dma_start(
                    eT[g][:],
                    et.ap()[:, 1024 * g : 1024 * (g + 1)].rearrange(
                        "(k p) q -> p k q", p=128
                    ),
                )
            wo_sb = const.tile([128, 2, D], bf16)
            nc.sync.dma_start(wo_sb[:], wot.ap().rearrange("p (k d) -> p k d", k=2))

            # --- constants ---
            ones = const.tile([128, 1], f32)
            nc.vector.memset(ones[:], 1.0)
            identb = const.tile([128, 128], bf16)
            make_identity(nc, identb[:])
            # warm the ACT exp table set early (~2.7us table load)
            warm = const.tile([128, 1], f32)
            nc.scalar.activation(warm[:], ones[:], EXP)
            # cvec broadcast (needed mid-stream for the nrm0 adds)
            cvst = const.tile([128, D], f32)
            nc.sync.dma_start(cvst[0:1, :], cvec.ap().unsqueeze(0))
            cvsb = const.tile([128, D], f32)
            nc.gpsimd.partition_broadcast(cvsb[:], cvst[0:1, :])

            qT = [big.tile([128, LQ], bf16, name=f"qT{h}") for h in range(2)]
            kT = [
                [big.tile([128, 1024], bf16, name=f"kT{h}_{g}") for g in range(4)]
                for h in range(2)
            ]
            v_g = [big.tile([128, 8, 256], bf16, name=f"v{g}") for g in range(4)]

            # --- projection units ---
            def unit_q(t):
                ps = psp.tile([128, 1024], f32, name=f"q_ps{t}", tag="sc")
                for c in range(2):
                    for k in range(4):
                        nc.tensor.matmul(
                            ps[:, 512 * c : 512 * c + 512],
                            wq_sb[:, k, 128 * t : 128 * t + 128],
                            xT[:, k, 512 * c : 512 * c + 512],
                            start=(k == 0),
                            stop=(k == 3),
                        )
                nc.scalar.activation(qT[t][:], ps[:], IDN, bias=bqsb[:, t : t + 1])

            def unit_k(h, g):
                ps = psp.tile([128, 1024], f32, name=f"k_ps{h}{g}", tag="sc")
                for c in range(2):
                    for k in range(4):
                        nc.tensor.matmul(
                            ps[:, 512 * c : 512 * c + 512],
                            wk_sb[:, k, 128 * h : 128 * h + 128],
                            eT[g][:, k, 512 * c : 512 * c + 512],
                            start=(k == 0),
                            stop=(k == 3),
                        )
                # h1's kT copies land in the ACT-slack h0 phase
                if h == 0:
                    nc.vector.tensor_copy(kT[h][g][:], ps[:])
                else:
                    nc.scalar.activation(kT[h][g][:], ps[:], IDN)

            def unit_v(g, pair):
                ps = psp.tile([128, 1024], f32, name=f"v_ps{g}{pair}", tag="sc")
                for w in range(2):
                    i = 2 * pair + w
                    for k in range(4):
                        nc.tensor.matmul(
                            ps[:, 256 * w : 256 * w + 256],
                            eT[g][:, k, 128 * i : 128 * i + 128],
                            wv_sb[:, k, :],
                            start=(k == 0),
                            stop=(k == 3),
                        )
                nc.vector.tensor_copy(
                    v_g[g][:, 2 * pair : 2 * pair + 2, :],
                    ps[:, 0:512].rearrange("p (w d) -> p w d", w=2),
                )

            # --- attention ---
            ctxT = big.tile([128, 2, LQ], bf16)
            att = {}
            recips = {}
            nrm0 = []
            out_ap = outp.ap().rearrange("(j p) e -> p j e", p=128)
            uid = [0]
            ESC = SCALE

            def _tr():
                uid[0] += 1
                return treep.tile([128, LQ], bf16, name=f"tr{uid[0]}", tag="tr")

            def _st(h):
                if h not in att:
                    att[h] = {
                        "ps_ctx": None,
                        "levels": [None] * 6,
                        "pend": [],
                        "run": None,
                        "npush": 0,
                    }
                return att[h]

            def tree_push(h, et_t):
                st = att[h]
                st["npush"] += 1
                if st["run"] is not None:
                    nxt = _tr()
                    nc.vector.tensor_add(nxt[:], st["run"][:], et_t[:])
                    st["run"] = nxt
                    return
                levels = st["levels"]
                cur, lvl = et_t, 0
                while levels[lvl] is not None:
                    nxt = _tr()
                    nc.vector.tensor_add(nxt[:], levels[lvl][:], cur[:])
                    levels[lvl] = None
                    cur, lvl = nxt, lvl + 1
                levels[lvl] = cur
                if st["npush"] == 25:
                    # collapse the tree into a running sum for a short tail
                    run = None
                    for l in range(6):
                        if levels[l] is None:
                            continue
                        if run is None:
                            run = levels[l]
                        else:
                            nxt = _tr()
                            nc.vector.tensor_add(nxt[:], run[:], levels[l][:])
                            run = nxt
                        levels[l] = None
                    st["run"] = run

            def emit_ctx_oldest(h, flush=False, maxpop=2, mindepth=2):
                st = _st(h)
                npop = 0
                while len(st["pend"]) > (0 if flush else mindepth) and (
                    flush or npop < maxpop
                ):
                    kt, et_t, g, i = st["pend"].pop(0)
                    npop += 1
                    if st["ps_ctx"] is None:
                        st["ps_ctx"] = ps_c.tile(
                            [128, LQ], f32, name=f"ctx{h}", tag="ctx"
                        )
                    for c in range(2):
                        nc.tensor.matmul(
                            st["ps_ctx"][:, 512 * c : 512 * c + 512],
                            v_g[g][:, i, 128 * h : 128 * h + 128],
                            et_t[:, 512 * c : 512 * c + 512],
                            start=(kt == 0),
                            stop=(kt == 31),
                        )
                    if kt != 31:
                        tree_push(h, et_t)
                    else:
                        st["last_et"] = et_t

            def score_exp(h, kt, pool):
                st = _st(h)
                g, i = kt // 8, kt % 8
                ps_sc = psp.tile([128, LQ], f32, name=f"sc{h}_{kt}", tag="sc")
                for c in range(2):
                    nc.tensor.matmul(
                        ps_sc[:, 512 * c : 512 * c + 512],
                        kT[h][g][:, 128 * i : 128 * i + 128],
                        qT[h][:, 512 * c : 512 * c + 512],
                        start=True,
                        stop=True,
                    )
                et_t = pool.tile([128, LQ], bf16, name=f"et{h}_{kt}", tag="et")
                nc.scalar.activation(et_t[:], ps_sc[:], EXP, scale=ESC)
                st["pend"].append((kt, et_t, g, i))

            def attn_step(h, kt):
                score_exp(h, kt, expp)
                emit_ctx_oldest(h)

            def finish_a(h):
                st = att[h]
                emit_ctx_oldest(h, flush=True)
                # ctxT halves first: they gate the tail out-projection MMs
                for c in range(2):
                    nc.vector.tensor_copy(
                        ctxT[:, h, 512 * c : 512 * c + 512],
                        st["ps_ctx"][:, 512 * c : 512 * c + 512],
                    )
                fin = _tr()
                for c in range(2):
                    nc.vector.tensor_add(
                        fin[:, 512 * c : 512 * c + 512],
                        st["run"][:, 512 * c : 512 * c + 512],
                        st["last_et"][:, 512 * c : 512 * c + 512],
                    )
                st["fin"] = fin

            def finish_b(h):
                st = att[h]
                fin = st["fin"]
                den = smal.tile([128, 8], f32, name=f"den{h}", tag="den")
                pt = psp.tile([128, LQ], bf16, name=f"dt{h}", tag="sc")
                for half in range(2):
                    for j in range(4):
                        jj = 4 * half + j
                        nc.tensor.transpose(
                            pt[:, 128 * jj : 128 * jj + 128],
                            fin[:, 128 * jj : 128 * jj + 128],
                            identb[:],
                        )
                    nc.vector.tensor_reduce(
                        den[:, 4 * half : 4 * half + 4],
                        pt[:, 512 * half : 512 * half + 512].rearrange(
                            "p (j q) -> p j q", j=4
                        ),
                        axis=mybir.AxisListType.X,
                        op=mybir.AluOpType.add,
                    )
                rc = smal.tile([128, 8], f32, name=f"rc{h}", tag="rc")
                nc.vector.reciprocal(rc[:], den[:])
                recips[h] = rc

            def outproj0(js):
                for j in js:
                    p = psp.tile([128, LQ], f32, name=f"o_ps0_{j}", tag="sc")
                    nc.tensor.matmul(
                        p[:, 0:512],
                        ctxT[:, 0, 128 * j : 128 * j + 128],
                        wo_sb[:, 0, :],
                        start=True,
                        stop=True,
                    )
                    n = nrm0p.tile([128, 512], f32, name=f"nrm0_{j}", tag="nrm0")
                    nc.vector.tensor_scalar_mul(
                        n[:], p[:, 0:512], recips[0][:, j : j + 1]
                    )
                    nc.vector.tensor_add(n[:], n[:], cvsb[:])
                    nrm0.append(n)

            def outproj1(js):
                for j in js:
                    p = psp.tile([128, LQ], f32, name=f"o_ps1_{j}", tag="sc")
                    nc.tensor.matmul(
                        p[:, 0:512],
                        ctxT[:, 1, 128 * j : 128 * j + 128],
                        wo_sb[:, 1, :],
                        start=True,
                        stop=True,
                    )
                    n1 = osb.tile([128, 512], f32, name=f"nrm1_{j}", tag="nrm1")
                    if j % 2 == 0:
                        nc.scalar.activation(
                            n1[:], p[:, 0:512], IDN, scale=recips[1][:, j : j + 1]
                        )
                    else:
                        nc.vector.tensor_scalar_mul(
                            n1[:], p[:, 0:512], recips[1][:, j : j + 1]
                        )
                    ob = osb.tile([128, 512], f32, name=f"ob{j}", tag="ob")
                    nc.vector.tensor_add(ob[:], nrm0[j][:], n1[:])
                    nc.sync.dma_start(out_ap[:, j, :], ob[:])

            # --- schedule ---
            inj = {}

            def add_inj(s, fn):
                inj.setdefault(s, []).append(fn)

            for gi, gn in enumerate((1, 2, 3)):
                base = 8 * gi
                add_inj(base + 0, lambda gn=gn: unit_k(0, gn))
                for pr in range(4):
                    add_inj(base + 1 + pr, lambda gn=gn, pr=pr: unit_v(gn, pr))
            add_inj(5, lambda: unit_k(1, 0))
            add_inj(13, lambda: unit_k(1, 1))
            add_inj(21, lambda: unit_k(1, 2))
            add_inj(26, lambda: unit_k(1, 3))
            # prefetch h1 kt0..7 score/exp into the late h0 phase
            for p in range(8):
                add_inj(24 + p, lambda p=p: score_exp(1, p, stashp))
            # finish_a(0) must be emitted BEFORE h1's first ctx matmul: ctx1's
            # PSUM buffer WAR-depends on ctx0's readers (the ctxT copies), and
            # the PE queue is strict FIFO.
            preinj = {32: [lambda: finish_a(0)]}
            add_inj(34, lambda: finish_b(0))
            # spread h0's out-projection: its 4 DVE ops per pair otherwise
            # congest the vector queue, delaying h1's tree pushes (et-pool
            # WAR -> exp stall -> PE gap)
            add_inj(36, lambda: outproj0([0, 1]))
            add_inj(40, lambda: outproj0([2, 3]))
            add_inj(44, lambda: outproj0([4, 5]))
            add_inj(48, lambda: outproj0([6, 7]))

            # pre-units: q projections + group-0 k/v
            unit_q(0)
            unit_q(1)
            unit_k(0, 0)
            unit_v(0, 0)
            unit_v(0, 1)
            unit_v(0, 2)
            unit_v(0, 3)

            for s in range(56):
                for fn in preinj.get(s, []):
                    fn()
                if s < 32:
                    attn_step(0, s)
                else:
                    attn_step(1, (s - 32) + 8)
                for fn in inj.get(s, []):
                    fn()

            finish_a(1)
            finish_b(1)
            outproj1(list(range(8)))

    nc.compile()
    return nc


def _get_nc():
    if "nc" not in _compiled:
        _compiled["nc"] = _build()
    return _compiled["nc"]


def _warr(wt, k, dtype_name="bfloat16", scale=1.0):
    """[k*128, n] -> [128, k*n] so partition p reads one contiguous block."""
    import ml_dtypes

    dt = getattr(ml_dtypes, dtype_name)
    n = wt.shape[1]
    return np.ascontiguousarray(
        (wt * scale).reshape(k, 128, n).transpose(1, 0, 2).reshape(128, k * n)
    ).astype(dt)


def _make_in_maps(x, encoder_feats, Wq, Wk, Wv, bq, bk, bv, Wo, bo):
    import ml_dtypes

    f = np.float32
    bf = ml_dtypes.bfloat16
    x = np.asarray(x, f)
    encoder_feats = np.asarray(encoder_feats, f)
    Wq, Wk, Wv, Wo = (np.asarray(a, f) for a in (Wq, Wk, Wv, Wo))
    bq, bk, bv, bo = (np.asarray(a, f) for a in (bq, bk, bv, bo))

    # host-side transpose + bf16 cast (one copy per batch)
    eT_b = [encoder_feats[b].T.astype(bf) for b in range(B)]  # [512, 4096] bf16
    xT_b = [x[b].T.astype(bf) for b in range(B)]  # [512, 1024] bf16

    # bk is dropped: adding bk to k shifts every score for a given query by the
    # same constant (q . bk), and softmax is invariant to that shift.
    per_hp = []
    for hp in range(2):
        sl = slice(256 * hp, 256 * hp + 256)
        cv = Wo[:, sl] @ bv[sl]
        if hp == 0:
            cv = cv + bo
        per_hp.append(
            {
                "wqt": _warr(Wq[sl, :].T, 4),
                "wkt": _warr(Wk[sl, :].T, 4),
                "wvt": _warr(Wv[sl, :].T, 4),
                "wot": _warr(Wo[:, sl].T, 2),
                "bq2": np.ascontiguousarray(bq[sl].reshape(2, 128).T, dtype=f),
                "cvec": np.ascontiguousarray(cv, dtype=f),
            }
        )

    in_maps = []
    for c in range(NCORES):
        b, hp = c // 2, c % 2
        m = {"et": eT_b[b], "xt": xT_b[b]}
        m.update(per_hp[hp])
        in_maps.append(m)
    return in_maps


def kernel(x, encoder_feats, Wq, Wk, Wv, bq, bk, bv, Wo, bo, _trace=False):
    from concourse.bass_utils import run_bass_kernel_spmd

    nc = _get_nc()
    in_maps = _make_in_maps(x, encoder_feats, Wq, Wk, Wv, bq, bk, bv, Wo, bo)
    kw = {}
    if _trace:
        kw = dict(trace=True, trace_cores=[0])
    res = run_bass_kernel_spmd(nc, in_maps, core_ids=list(range(NCORES)), **kw)
    _compiled["last_res"] = res
    out = np.empty((B, LQ, D), np.float32)
    for b in range(B):
        out[b] = res.results[2 * b]["outp"] + res.results[2 * b + 1]["outp"]
    return out
